# revision 8
# baseline (speedup 1.0000x reference)
"""Multi-head cross-attention Trainium2 kernel.

Data-parallel over batch B=8: one NeuronCore per batch episode.
All tensors per-core are small enough that the whole problem lives in SBUF.

Math (per batch b, fp32):
  f1 -> tokens X1 [L1=720, C=512] (feature-major in DRAM already: [n, C, HW])
  f2 -> tokens X2 [L2=2160, C=512]
  Q = X2 @ Wq + bq ; K = X1 @ Wk + bk ; V = X1 @ Wv + bv
  P = exp(Q K^T * scale)  (no max subtraction; scores are O(1))
  Z = rowsum(P); att = P / Z
  out2 = (att @ V) @ Woq + boq   -> [L2, C] -> untokens
  out1 = (att^T @ Q) @ Wok + bok -> [L1, C] -> untokens

Layouts on core:
  "fm" = feature-major [C(part chunks of 128), L] ; "tm" = token-major [L(part), C]
  Scores pass A: S[q_tile 128, k 720] tm -> exp (+row sums via ACT accum) -> P
  out1: accumulate over q tiles: out1_h^T[d, k] = sum_t Q'_t^T @ P_t (Q' = Q/Z)
  Scores pass B: S^T[k_tile 128, q 432] recomputed via K@Q^T, exp -> PT
  out2: out2_h^T[d, q] 432-chunks = sum_kt V_kt^T... (lhsT=V tm, rhs=PT), / Z via
        row-broadcast of 1/Z (built once per head via a DRAM bounce + bcast DMA)
"""

import numpy as np

B = 8
C = 512
NH = 4
D = 128  # head dim == one C-chunk
N1, N2, F = 5, 15, 144
L1, L2 = N1 * F, N2 * F  # 720, 2160
SCALE = float(D) ** -0.5

QT = 17  # ceil(2160/128); last tile 112 rows
KT = 6   # ceil(720/128); last tile 80 rows
QC = 5   # 2160 = 5*432 free-dim chunks
KC = 2   # 720 = 2*360 free-dim chunks

_CACHE = {}


def _qq(t):
    return 128 if t < QT - 1 else L2 - 128 * (QT - 1)  # 112


def _kk(t):
    return 128 if t < KT - 1 else L1 - 128 * (KT - 1)  # 80


def _build():
    from contextlib import ExitStack

    import concourse.bass as bass
    import concourse.mybir as mybir
    import concourse.tile as tile
    from concourse.masks import make_identity

    fp32 = mybir.dt.float32
    f32r = mybir.dt.float32r
    EXP = mybir.ActivationFunctionType.Exp
    COPY = mybir.ActivationFunctionType.Copy
    IDENT = mybir.ActivationFunctionType.Identity

    def r(ap):  # bitcast fp32 -> float32r for full-rate PE matmul
        return ap.bitcast(f32r)

    nc = bass.Bass()

    f1_d = nc.dram_tensor("f1", [N1, C, F], fp32, kind="ExternalInput")
    f2_d = nc.dram_tensor("f2", [N2, C, F], fp32, kind="ExternalInput")
    w_d = {}
    b_d = {}
    for w in ("q", "k", "v", "oq", "ok"):
        w_d[w] = nc.dram_tensor(f"W{w}", [C, C], fp32, kind="ExternalInput")
        b_d[w] = nc.dram_tensor(f"b{w}", [C], fp32, kind="ExternalInput")
    out1_d = nc.dram_tensor("out1", [N1, C, F], fp32, kind="ExternalOutput")
    out2_d = nc.dram_tensor("out2", [N2, C, F], fp32, kind="ExternalOutput")
    scratch_d = nc.dram_tensor("rrow_scratch", [NH, QT * 128], fp32)

    with ExitStack() as ctx:
        tc = ctx.enter_context(tile.TileContext(nc))

        consts = ctx.enter_context(tc.tile_pool(name="consts", bufs=1))
        acts = ctx.enter_context(tc.tile_pool(name="acts", bufs=1))
        ps_pool = ctx.enter_context(tc.tile_pool(name="ps", bufs=4, space="PSUM"))
        acc_pool = ctx.enter_context(tc.tile_pool(name="acc", bufs=2, space="PSUM"))

        # ---- constants ----
        wt = {}
        bf = {}
        for w in ("q", "k", "v", "oq", "ok"):
            wt[w] = consts.tile([128, 4, C], fp32, tag=f"W{w}", name=f"W{w}_t")
            nc.sync.dma_start(out=wt[w], in_=w_d[w].rearrange("(k p) n -> p k n", p=128))
        for w in ("q", "k", "oq", "ok"):
            bf[w] = consts.tile([128, 4], fp32, tag=f"b{w}", name=f"b{w}_f")
            nc.sync.dma_start(out=bf[w], in_=b_d[w].rearrange("(m p) -> p m", p=128))
        bv_bc = consts.tile([128, C], fp32, tag="bv")
        nc.sync.dma_start(out=bv_bc, in_=b_d["v"][:].partition_broadcast(128))
        ident = consts.tile([128, 128], fp32, tag="ident")
        make_identity(nc, ident)

        # ---- phase 1: load X, project Q/K/V ----
        with tc.tile_pool(name="x", bufs=1) as xp:
            x2 = xp.tile([128, 4, L2], fp32, tag="x2")
            x1 = xp.tile([128, 4, L1], fp32, tag="x1")
            for c in range(4):
                nc.sync.dma_start(
                    out=x2[:, c, :].rearrange("p (n f) -> p n f", f=F),
                    in_=f2_d[:, c * 128 : (c + 1) * 128, :].rearrange("n p f -> p n f"),
                )
                nc.sync.dma_start(
                    out=x1[:, c, :].rearrange("p (n f) -> p n f", f=F),
                    in_=f1_d[:, c * 128 : (c + 1) * 128, :].rearrange("n p f -> p n f"),
                )

            qfm = acts.tile([128, 4, L2], fp32, tag="qfm")
            kfm = acts.tile([128, 4, L1], fp32, tag="kfm")
            vtm = acts.tile([128, KT, C], fp32, tag="vtm")

            for m in range(4):
                for qc in range(QC):
                    sl = slice(qc * 432, (qc + 1) * 432)
                    ps = ps_pool.tile([128, 432], fp32, tag="ps")
                    for k in range(4):
                        nc.tensor.matmul(
                            ps,
                            lhsT=r(wt["q"][:, k, m * 128 : (m + 1) * 128]),
                            rhs=r(x2[:, k, sl]),
                            start=(k == 0),
                            stop=(k == 3),
                        )
                    nc.scalar.activation(
                        qfm[:, m, sl], ps, IDENT, bias=bf["q"][:, m : m + 1]
                    )
                for kc in range(KC):
                    sl = slice(kc * 360, (kc + 1) * 360)
                    ps = ps_pool.tile([128, 432], fp32, tag="ps")
                    for k in range(4):
                        nc.tensor.matmul(
                            ps[:, 0:360],
                            lhsT=r(wt["k"][:, k, m * 128 : (m + 1) * 128]),
                            rhs=r(x1[:, k, sl]),
                            start=(k == 0),
                            stop=(k == 3),
                        )
                    nc.scalar.activation(
                        kfm[:, m, sl], ps[:, 0:360], IDENT, bias=bf["k"][:, m : m + 1]
                    )
            for kt in range(KT):
                kk = _kk(kt)
                ps = ps_pool.tile([128, 512], fp32, tag="ps")
                for k in range(4):
                    nc.tensor.matmul(
                        ps[0:kk, :],
                        lhsT=r(x1[:, k, kt * 128 : kt * 128 + kk]),
                        rhs=r(wt["v"][:, k, :]),
                        start=(k == 0),
                        stop=(k == 3),
                    )
                nc.vector.tensor_add(vtm[0:kk, kt, :], ps[0:kk, :], bv_bc[0:kk, :])

        # ---- phase 2: attention per head ----
        out1fm = acts.tile([128, 4, L1], fp32, tag="out1fm")
        out2fm = acts.tile([128, 4, L2], fp32, tag="out2fm")
        r_all = acts.tile([128, NH, QT], fp32, tag="r_all")
        nc.vector.memset(r_all, 0.0)

        ctx2 = ExitStack()
        p_pool = ctx2.enter_context(tc.tile_pool(name="p", bufs=2))
        pt_pool = ctx2.enter_context(tc.tile_pool(name="pt", bufs=8))
        qp_pool = ctx2.enter_context(tc.tile_pool(name="qp", bufs=2))
        rrow_pool = ctx2.enter_context(tc.tile_pool(name="rrow", bufs=1))
        small = ctx2.enter_context(tc.tile_pool(name="small", bufs=6))
        for h in range(NH):
            hs = slice(h * 128, (h + 1) * 128)
            acc = [acc_pool.tile([128, 360], fp32, tag="acc", name=f"acc{i}") for i in range(KC)]
            for t in range(QT):
                qq = _qq(t)
                q0 = t * 128
                p_t = p_pool.tile([128, L1], fp32, tag="p")
                z0 = small.tile([128, 1], fp32, tag="z")
                z1 = small.tile([128, 1], fp32, tag="z")
                for half in range(KC):
                    sl = slice(half * 360, (half + 1) * 360)
                    ps = ps_pool.tile([128, 432], fp32, tag="ps")
                    nc.tensor.matmul(
                        ps[0:qq, 0:360],
                        lhsT=r(qfm[:, h, q0 : q0 + qq]),
                        rhs=r(kfm[:, h, sl]),
                        start=True,
                        stop=True,
                    )
                    nc.scalar.activation(
                        p_t[0:qq, sl],
                        ps[0:qq, 0:360],
                        EXP,
                        scale=SCALE,
                        accum_out=(z0 if half == 0 else z1)[0:qq, :],
                    )
                nc.vector.tensor_add(z0[0:qq, :], z0[0:qq, :], z1[0:qq, :])
                r_t = r_all[:, h, t : t + 1]
                nc.vector.reciprocal(r_t[0:qq, :], z0[0:qq, :])
                # Q' = (Q tile)^T scaled by 1/Z
                pq = ps_pool.tile([128, 432], fp32, tag="ps")
                nc.tensor.transpose(pq[0:qq, 0:128], qfm[:, h, q0 : q0 + qq], ident)
                qp = qp_pool.tile([128, 128], fp32, tag="qp")
                nc.scalar.activation(
                    qp[0:qq, :], pq[0:qq, 0:128], COPY, scale=r_t[0:qq, :]
                )
                for half in range(KC):
                    sl = slice(half * 360, (half + 1) * 360)
                    nc.tensor.matmul(
                        acc[half],
                        lhsT=r(qp[0:qq, :]),
                        rhs=r(p_t[0:qq, sl]),
                        start=(t == 0),
                        stop=(t == QT - 1),
                    )
            for half in range(KC):
                nc.scalar.copy(out1fm[:, h, half * 360 : (half + 1) * 360], acc[half])

            # 1/Z as a row vector broadcast across partitions (DRAM bounce)
            nc.sync.dma_start(
                out=scratch_d[h].rearrange("(t p) -> p t", p=128), in_=r_all[:, h, :]
            )
            rrow = rrow_pool.tile([128, L2], fp32, tag="rrow")
            nc.sync.dma_start(out=rrow, in_=scratch_d[h][0:L2].partition_broadcast(128))

            for qc in range(QC):
                sl = slice(qc * 432, (qc + 1) * 432)
                pts = []
                for kt in range(KT):
                    kk = _kk(kt)
                    ps = ps_pool.tile([128, 432], fp32, tag="ps")
                    nc.tensor.matmul(
                        ps[0:kk, :],
                        lhsT=r(kfm[:, h, kt * 128 : kt * 128 + kk]),
                        rhs=r(qfm[:, h, sl]),
                        start=True,
                        stop=True,
                    )
                    ptt = pt_pool.tile([128, 432], fp32, tag="pt", name=f"pt{kt}")
                    nc.scalar.activation(ptt[0:kk, :], ps[0:kk, :], EXP, scale=SCALE)
                    pts.append(ptt)
                po = ps_pool.tile([128, 432], fp32, tag="ps")
                for kt in range(KT):
                    kk = _kk(kt)
                    nc.tensor.matmul(
                        po,
                        lhsT=r(vtm[0:kk, kt, hs]),
                        rhs=r(pts[kt][0:kk, :]),
                        start=(kt == 0),
                        stop=(kt == KT - 1),
                    )
                nc.vector.tensor_mul(out2fm[:, h, sl], po, rrow[:, sl])

        # ---- phase 3: output projections ----
        ctx2.close()
        stage_pool = ctx.enter_context(tc.tile_pool(name="stage", bufs=3))
        fin1 = acts.tile([128, 4, L1], fp32, tag="fin1")
        for m in range(4):
            for qc in range(QC):
                sl = slice(qc * 432, (qc + 1) * 432)
                ps = ps_pool.tile([128, 432], fp32, tag="ps")
                for k in range(4):
                    nc.tensor.matmul(
                        ps,
                        lhsT=r(wt["oq"][:, k, m * 128 : (m + 1) * 128]),
                        rhs=r(out2fm[:, k, sl]),
                        start=(k == 0),
                        stop=(k == 3),
                    )
                stage = stage_pool.tile([128, 432], fp32, tag="stage")
                nc.scalar.activation(
                    stage, ps, IDENT, bias=bf["oq"][:, m : m + 1]
                )
                nc.sync.dma_start(
                    out=out2_d[3 * qc : 3 * qc + 3, m * 128 : (m + 1) * 128, :].rearrange(
                        "n p f -> p n f"
                    ),
                    in_=stage.rearrange("p (n f) -> p n f", f=F),
                )
            for kc in range(KC):
                sl = slice(kc * 360, (kc + 1) * 360)
                ps = ps_pool.tile([128, 432], fp32, tag="ps")
                for k in range(4):
                    nc.tensor.matmul(
                        ps[:, 0:360],
                        lhsT=r(wt["ok"][:, k, m * 128 : (m + 1) * 128]),
                        rhs=r(out1fm[:, k, sl]),
                        start=(k == 0),
                        stop=(k == 3),
                    )
                nc.scalar.activation(
                    fin1[:, m, sl], ps[:, 0:360], IDENT, bias=bf["ok"][:, m : m + 1]
                )
            nc.sync.dma_start(
                out=out1_d[:, m * 128 : (m + 1) * 128, :].rearrange("n p f -> p n f"),
                in_=fin1[:, m, :].rearrange("p (n f) -> p n f", f=F),
            )

    return nc


def get_nc():
    if "nc" not in _CACHE:
        _CACHE["nc"] = _build()
    return _CACHE["nc"]


def kernel(f1, f2, Wq, bq, Wk, bk, Wv, bv, Woq, boq, Wok, bok, **run_kwargs):
    from concourse.bass_utils import run_bass_kernel_spmd

    nc = get_nc()
    f1 = np.ascontiguousarray(np.asarray(f1, dtype=np.float32)).reshape(B, N1, C, F)
    f2 = np.ascontiguousarray(np.asarray(f2, dtype=np.float32)).reshape(B, N2, C, F)
    ws = {
        "Wq": Wq, "bq": bq, "Wk": Wk, "bk": bk, "Wv": Wv, "bv": bv,
        "Woq": Woq, "boq": boq, "Wok": Wok, "bok": bok,
    }
    ws = {k: np.ascontiguousarray(np.asarray(v, dtype=np.float32)) for k, v in ws.items()}
    in_maps = [{"f1": f1[i], "f2": f2[i], **ws} for i in range(B)]
    res = run_bass_kernel_spmd(nc, in_maps, core_ids=list(range(B)), **run_kwargs)
    results = res.results if hasattr(res, "results") else res
    out1 = np.stack([results[i]["out1"].reshape(N1, C, 12, 12) for i in range(B)])
    out2 = np.stack([results[i]["out2"].reshape(N2, C, 12, 12) for i in range(B)])
    if run_kwargs:
        return (out1, out2), res
    return (out1, out2)


# revision 9
# speedup vs baseline: 1.0968x; 1.0968x over previous
"""Multi-head cross-attention Trainium2 kernel.

Data-parallel over batch B=8: one NeuronCore per batch episode.
All tensors per-core are small enough that the whole problem lives in SBUF.

Math (per batch b, fp32):
  f1 -> tokens X1 [L1=720, C=512] (feature-major in DRAM already: [n, C, HW])
  f2 -> tokens X2 [L2=2160, C=512]
  Q = X2 @ Wq + bq ; K = X1 @ Wk + bk ; V = X1 @ Wv + bv
  P = exp(Q K^T * scale)  (no max subtraction; scores are O(1))
  Z = rowsum(P); att = P / Z
  out2 = (att @ V) @ Woq + boq   -> [L2, C] -> untokens
  out1 = (att^T @ Q) @ Wok + bok -> [L1, C] -> untokens

Layouts on core:
  "fm" = feature-major [C(part chunks of 128), L] ; "tm" = token-major [L(part), C]
  Scores pass A: S[q_tile 128, k 720] tm -> exp (+row sums via ACT accum) -> P
  out1: accumulate over q tiles: out1_h^T[d, k] = sum_t Q'_t^T @ P_t (Q' = Q/Z)
  Scores pass B: S^T[k_tile 128, q 432] recomputed via K@Q^T, exp -> PT
  out2: out2_h^T[d, q] 432-chunks = sum_kt V_kt^T... (lhsT=V tm, rhs=PT), / Z via
        row-broadcast of 1/Z (built once per head via a DRAM bounce + bcast DMA)
"""

import numpy as np

B = 8
C = 512
NH = 4
D = 128  # head dim == one C-chunk
N1, N2, F = 5, 15, 144
L1, L2 = N1 * F, N2 * F  # 720, 2160
SCALE = float(D) ** -0.5

QT = 17  # ceil(2160/128); last tile 112 rows
KT = 6   # ceil(720/128); last tile 80 rows
QC = 5   # 2160 = 5*432 free-dim chunks
KC = 2   # 720 = 2*360 free-dim chunks

_CACHE = {}


def _qq(t):
    return 128 if t < QT - 1 else L2 - 128 * (QT - 1)  # 112


def _kk(t):
    return 128 if t < KT - 1 else L1 - 128 * (KT - 1)  # 80


def _build():
    from contextlib import ExitStack

    import concourse.bass as bass
    import concourse.mybir as mybir
    import concourse.tile as tile
    from concourse.masks import make_identity

    fp32 = mybir.dt.float32
    f32r = mybir.dt.float32r
    EXP = mybir.ActivationFunctionType.Exp
    COPY = mybir.ActivationFunctionType.Copy
    IDENT = mybir.ActivationFunctionType.Identity

    def r(ap):  # bitcast fp32 -> float32r for full-rate PE matmul
        return ap.bitcast(f32r)

    nc = bass.Bass()

    f1_d = nc.dram_tensor("f1", [N1, C, F], fp32, kind="ExternalInput")
    f2_d = nc.dram_tensor("f2", [N2, C, F], fp32, kind="ExternalInput")
    w_d = {}
    b_d = {}
    for w in ("q", "k", "v", "oq", "ok"):
        w_d[w] = nc.dram_tensor(f"W{w}", [C, C], fp32, kind="ExternalInput")
        b_d[w] = nc.dram_tensor(f"b{w}", [C], fp32, kind="ExternalInput")
    out1_d = nc.dram_tensor("out1", [N1, C, F], fp32, kind="ExternalOutput")
    out2_d = nc.dram_tensor("out2", [N2, C, F], fp32, kind="ExternalOutput")
    scratch_d = nc.dram_tensor("rrow_scratch", [NH, QT * 128], fp32)

    with ExitStack() as ctx:
        tc = ctx.enter_context(tile.TileContext(nc))

        consts = ctx.enter_context(tc.tile_pool(name="consts", bufs=1))
        acts = ctx.enter_context(tc.tile_pool(name="acts", bufs=1))
        ps_pool = ctx.enter_context(tc.tile_pool(name="ps", bufs=4, space="PSUM"))
        acc_pool = ctx.enter_context(tc.tile_pool(name="acc", bufs=2, space="PSUM"))

        # ---- constants ----
        wt = {}
        bf = {}
        for w in ("q", "k", "v", "oq", "ok"):
            wt[w] = consts.tile([128, 4, C], fp32, tag=f"W{w}", name=f"W{w}_t")
            nc.sync.dma_start(out=wt[w], in_=w_d[w].rearrange("(k p) n -> p k n", p=128))
        for w in ("q", "k", "oq", "ok"):
            bf[w] = consts.tile([128, 4], fp32, tag=f"b{w}", name=f"b{w}_f")
            nc.sync.dma_start(out=bf[w], in_=b_d[w].rearrange("(m p) -> p m", p=128))
        bv_bc = consts.tile([128, C], fp32, tag="bv")
        nc.sync.dma_start(out=bv_bc, in_=b_d["v"][:].partition_broadcast(128))
        ident = consts.tile([128, 128], fp32, tag="ident")
        make_identity(nc, ident)

        # ---- phase 1: load X, project Q/K/V ----
        with tc.tile_pool(name="x", bufs=1) as xp:
            x2 = xp.tile([128, 4, L2], fp32, tag="x2")
            x1 = xp.tile([128, 4, L1], fp32, tag="x1")
            for c in range(4):
                nc.sync.dma_start(
                    out=x2[:, c, :].rearrange("p (n f) -> p n f", f=F),
                    in_=f2_d[:, c * 128 : (c + 1) * 128, :].rearrange("n p f -> p n f"),
                )
                nc.sync.dma_start(
                    out=x1[:, c, :].rearrange("p (n f) -> p n f", f=F),
                    in_=f1_d[:, c * 128 : (c + 1) * 128, :].rearrange("n p f -> p n f"),
                )

            qfm = acts.tile([128, 4, L2], fp32, tag="qfm")
            kfm = acts.tile([128, 4, L1], fp32, tag="kfm")
            vtm = acts.tile([128, KT, C], fp32, tag="vtm")

            for m in range(4):
                for qc in range(QC):
                    sl = slice(qc * 432, (qc + 1) * 432)
                    ps = ps_pool.tile([128, 432], fp32, tag="ps")
                    for k in range(4):
                        nc.tensor.matmul(
                            ps,
                            lhsT=r(wt["q"][:, k, m * 128 : (m + 1) * 128]),
                            rhs=r(x2[:, k, sl]),
                            start=(k == 0),
                            stop=(k == 3),
                        )
                    nc.vector.tensor_scalar_add(qfm[:, m, sl], ps, bf["q"][:, m : m + 1])
                for kc in range(KC):
                    sl = slice(kc * 360, (kc + 1) * 360)
                    ps = ps_pool.tile([128, 432], fp32, tag="ps")
                    for k in range(4):
                        nc.tensor.matmul(
                            ps[:, 0:360],
                            lhsT=r(wt["k"][:, k, m * 128 : (m + 1) * 128]),
                            rhs=r(x1[:, k, sl]),
                            start=(k == 0),
                            stop=(k == 3),
                        )
                    nc.vector.tensor_scalar_add(
                        kfm[:, m, sl], ps[:, 0:360], bf["k"][:, m : m + 1]
                    )
            for kt in range(KT):
                kk = _kk(kt)
                ps = ps_pool.tile([128, 512], fp32, tag="ps")
                for k in range(4):
                    nc.tensor.matmul(
                        ps[0:kk, :],
                        lhsT=r(x1[:, k, kt * 128 : kt * 128 + kk]),
                        rhs=r(wt["v"][:, k, :]),
                        start=(k == 0),
                        stop=(k == 3),
                    )
                nc.vector.tensor_add(vtm[0:kk, kt, :], ps[0:kk, :], bv_bc[0:kk, :])

        # ---- phase 2: attention per head ----
        out1fm = acts.tile([128, 4, L1], fp32, tag="out1fm")
        out2fm = acts.tile([128, 4, L2], fp32, tag="out2fm")
        r_all = acts.tile([128, NH, QT], fp32, tag="r_all")
        nc.vector.memset(r_all, 0.0)

        ctx2 = ExitStack()
        p_pool = ctx2.enter_context(tc.tile_pool(name="p", bufs=2))
        pt_pool = ctx2.enter_context(tc.tile_pool(name="pt", bufs=8))
        qp_pool = ctx2.enter_context(tc.tile_pool(name="qp", bufs=2))
        rrow_pool = ctx2.enter_context(tc.tile_pool(name="rrow", bufs=1))
        small = ctx2.enter_context(tc.tile_pool(name="small", bufs=6))
        for h in range(NH):
            hs = slice(h * 128, (h + 1) * 128)
            acc = [acc_pool.tile([128, 360], fp32, tag="acc", name=f"acc{i}") for i in range(KC)]
            for t in range(QT):
                qq = _qq(t)
                q0 = t * 128
                p_t = p_pool.tile([128, L1], fp32, tag="p")
                z0 = small.tile([128, 1], fp32, tag="z")
                z1 = small.tile([128, 1], fp32, tag="z")
                for half in range(KC):
                    sl = slice(half * 360, (half + 1) * 360)
                    ps = ps_pool.tile([128, 432], fp32, tag="ps")
                    nc.tensor.matmul(
                        ps[0:qq, 0:360],
                        lhsT=r(qfm[:, h, q0 : q0 + qq]),
                        rhs=r(kfm[:, h, sl]),
                        start=True,
                        stop=True,
                    )
                    nc.scalar.activation(
                        p_t[0:qq, sl],
                        ps[0:qq, 0:360],
                        EXP,
                        scale=SCALE,
                        accum_out=(z0 if half == 0 else z1)[0:qq, :],
                    )
                nc.vector.tensor_add(z0[0:qq, :], z0[0:qq, :], z1[0:qq, :])
                r_t = r_all[:, h, t : t + 1]
                nc.vector.reciprocal(r_t[0:qq, :], z0[0:qq, :])
                # Q' = (Q tile)^T scaled by 1/Z
                pq = ps_pool.tile([128, 432], fp32, tag="ps")
                nc.tensor.transpose(pq[0:qq, 0:128], qfm[:, h, q0 : q0 + qq], ident)
                qp = qp_pool.tile([128, 128], fp32, tag="qp")
                nc.vector.tensor_scalar_mul(qp[0:qq, :], pq[0:qq, 0:128], r_t[0:qq, :])
                for half in range(KC):
                    sl = slice(half * 360, (half + 1) * 360)
                    nc.tensor.matmul(
                        acc[half],
                        lhsT=r(qp[0:qq, :]),
                        rhs=r(p_t[0:qq, sl]),
                        start=(t == 0),
                        stop=(t == QT - 1),
                    )
            for half in range(KC):
                nc.vector.tensor_copy(out1fm[:, h, half * 360 : (half + 1) * 360], acc[half])

            # 1/Z as a row vector broadcast across partitions (DRAM bounce)
            nc.sync.dma_start(
                out=scratch_d[h].rearrange("(t p) -> p t", p=128), in_=r_all[:, h, :]
            )
            rrow = rrow_pool.tile([128, L2], fp32, tag="rrow")
            nc.sync.dma_start(out=rrow, in_=scratch_d[h][0:L2].partition_broadcast(128))

            for qc in range(QC):
                sl = slice(qc * 432, (qc + 1) * 432)
                pts = []
                for kt in range(KT):
                    kk = _kk(kt)
                    ps = ps_pool.tile([128, 432], fp32, tag="ps")
                    nc.tensor.matmul(
                        ps[0:kk, :],
                        lhsT=r(kfm[:, h, kt * 128 : kt * 128 + kk]),
                        rhs=r(qfm[:, h, sl]),
                        start=True,
                        stop=True,
                    )
                    ptt = pt_pool.tile([128, 432], fp32, tag="pt", name=f"pt{kt}")
                    nc.scalar.activation(ptt[0:kk, :], ps[0:kk, :], EXP, scale=SCALE)
                    pts.append(ptt)
                po = ps_pool.tile([128, 432], fp32, tag="ps")
                for kt in range(KT):
                    kk = _kk(kt)
                    nc.tensor.matmul(
                        po,
                        lhsT=r(vtm[0:kk, kt, hs]),
                        rhs=r(pts[kt][0:kk, :]),
                        start=(kt == 0),
                        stop=(kt == KT - 1),
                    )
                nc.vector.tensor_mul(out2fm[:, h, sl], po, rrow[:, sl])

        # ---- phase 3: output projections ----
        ctx2.close()
        stage_pool = ctx.enter_context(tc.tile_pool(name="stage", bufs=3))
        fin1 = acts.tile([128, 4, L1], fp32, tag="fin1")
        for m in range(4):
            for qc in range(QC):
                sl = slice(qc * 432, (qc + 1) * 432)
                ps = ps_pool.tile([128, 432], fp32, tag="ps")
                for k in range(4):
                    nc.tensor.matmul(
                        ps,
                        lhsT=r(wt["oq"][:, k, m * 128 : (m + 1) * 128]),
                        rhs=r(out2fm[:, k, sl]),
                        start=(k == 0),
                        stop=(k == 3),
                    )
                stage = stage_pool.tile([128, 432], fp32, tag="stage")
                nc.vector.tensor_scalar_add(stage, ps, bf["oq"][:, m : m + 1])
                nc.sync.dma_start(
                    out=out2_d[3 * qc : 3 * qc + 3, m * 128 : (m + 1) * 128, :].rearrange(
                        "n p f -> p n f"
                    ),
                    in_=stage.rearrange("p (n f) -> p n f", f=F),
                )
            for kc in range(KC):
                sl = slice(kc * 360, (kc + 1) * 360)
                ps = ps_pool.tile([128, 432], fp32, tag="ps")
                for k in range(4):
                    nc.tensor.matmul(
                        ps[:, 0:360],
                        lhsT=r(wt["ok"][:, k, m * 128 : (m + 1) * 128]),
                        rhs=r(out1fm[:, k, sl]),
                        start=(k == 0),
                        stop=(k == 3),
                    )
                nc.vector.tensor_scalar_add(
                    fin1[:, m, sl], ps[:, 0:360], bf["ok"][:, m : m + 1]
                )
            nc.sync.dma_start(
                out=out1_d[:, m * 128 : (m + 1) * 128, :].rearrange("n p f -> p n f"),
                in_=fin1[:, m, :].rearrange("p (n f) -> p n f", f=F),
            )

    return nc


def get_nc():
    if "nc" not in _CACHE:
        _CACHE["nc"] = _build()
    return _CACHE["nc"]


def kernel(f1, f2, Wq, bq, Wk, bk, Wv, bv, Woq, boq, Wok, bok, **run_kwargs):
    from concourse.bass_utils import run_bass_kernel_spmd

    nc = get_nc()
    f1 = np.ascontiguousarray(np.asarray(f1, dtype=np.float32)).reshape(B, N1, C, F)
    f2 = np.ascontiguousarray(np.asarray(f2, dtype=np.float32)).reshape(B, N2, C, F)
    ws = {
        "Wq": Wq, "bq": bq, "Wk": Wk, "bk": bk, "Wv": Wv, "bv": bv,
        "Woq": Woq, "boq": boq, "Wok": Wok, "bok": bok,
    }
    ws = {k: np.ascontiguousarray(np.asarray(v, dtype=np.float32)) for k, v in ws.items()}
    in_maps = [{"f1": f1[i], "f2": f2[i], **ws} for i in range(B)]
    res = run_bass_kernel_spmd(nc, in_maps, core_ids=list(range(B)), **run_kwargs)
    results = res.results if hasattr(res, "results") else res
    out1 = np.stack([results[i]["out1"].reshape(N1, C, 12, 12) for i in range(B)])
    out2 = np.stack([results[i]["out2"].reshape(N2, C, 12, 12) for i in range(B)])
    if run_kwargs:
        return (out1, out2), res
    return (out1, out2)
